# revision 10
# baseline (speedup 1.0000x reference)
"""Trainium2 Bass kernel for a 2-layer cosine-similarity attention GCN.

Reference math (per (b,h) slice, two chained blocks):
    xn = x / max(||x||_row, eps)
    A  = softmax((xn @ xn^T) / max(alpha, 0.01), axis=-1)
    out = relu((A @ x) @ W^T + x)

Shapes: x [4, 4, 4096, 64] fp32; W [64, 64]. B*H = 16 slices sharded as
2 slices per NeuronCore across 8 cores (fully independent, no collectives).

Architecture (per core, 2 pairs x 2 blocks, all on-chip):
  - (A @ x) @ W^T == A @ (x @ W^T): W is folded into the value matrix up
    front, so the per-chunk epilogue is just transpose + rescale + residual.
  - Softmax without max-subtraction (logits are cosine sims * scale):
    E = exp(s*scale - scale) in (0, 1]; Z arrives via a ones-column in the
    value pack; division by Z is applied after the epilogue transpose.
  - S-matmuls run with K=128 at full PE stream rate (0.42ns/col) by packing
    both pairs' normalized-transposed rows into one 128-row stationary
    (rows 0-63 pair0, 64-127 pair1) and zero-padding the inactive pair's
    rows of the moving operand.  K=64 matmuls stream at only half rate, so
    this doubles S throughput.
  - U-matmuls (E^T-weighted value sums) use fp8e4 DoubleRow: two J-tiles
    (contraction 256) per instruction at 0.42ns/col -> 2x over bf16.
    Value pack m = [x@W^T, 1, 0-pad] is 80 wide (DoubleRow needs the slot
    stride %16 == 0 and an even width).
  - exp is split between the scalar engine (ACT, fp8e4 out) and the vector
    engine: DVE computes exp via a one-op Schraudolph trick (y = s*A + B
    converted to int16 == bf16 bit pattern of 2^(log2e*(s*scale-scale))),
    emitting bf16 E tiles consumed by plain bf16 U-matmuls. This roughly
    halves the scalar-engine wall which otherwise bottlenecks the kernel.
  - row 1/||x|| uses a fast inverse sqrt (bit trick + 3 Newton steps) on the
    vector engine so the scalar engine only ever runs Exp.
"""

import numpy as np

import concourse.bacc as bacc
import concourse.tile as tile
from concourse import mybir
from concourse.bass import broadcast_tensor_aps
from concourse.bass_utils import run_bass_kernel_spmd
from concourse.masks import make_identity

F32 = mybir.dt.float32
U32 = mybir.dt.uint32
I32 = mybir.dt.int32
I16 = mybir.dt.int16
BF16 = mybir.dt.bfloat16
FP8 = mybir.dt.float8e4
AF = mybir.ActivationFunctionType
ALU = mybir.AluOpType
PM = mybir.MatmulPerfMode

P = 128
D = 64
N_CORES = 8
MW = 80  # m-pack width: 64 values + 1 ones + 15 zero pad (DoubleRow align)
LOG2E = 1.4426950408889634
# Schraudolph correction picked to roughly center the relative error of
# 2^f ~ linear-in-mantissa; in bf16-bit units (128 per octave).
SCHRAUD_C = -9.3

# J-pair indices whose exp runs on the DVE (bf16 E) instead of ACT (fp8 E).
DVE_JP = (2, 6, 10, 14)


def build_nc(scales, n_rows=4096, npairs=2):
    assert npairs == 2, "kernel assumes 2 (b,h) slices per core"
    nblocks = len(scales)
    NT = n_rows // P          # 32 row tiles per pair
    CHW = 1024                # i-chunk width (ACT call width, 2 PSUM banks)
    NCH = n_rows // CHW       # 4 chunks
    HALF = 512                # matmul moving-dim limit
    NH = CHW // HALF
    NJP = NT // 2             # 16 J-pairs

    nc = bacc.Bacc("TRN2", target_bir_lowering=False, debug=False, num_devices=N_CORES)
    xin = nc.dram_tensor("xin", [npairs, n_rows, D], F32, kind="ExternalInput").ap()
    wts = [
        nc.dram_tensor(f"w{i}t", [D, D], F32, kind="ExternalInput").ap()
        for i in range(nblocks)
    ]
    out = nc.dram_tensor("out", [npairs, n_rows, D], F32, kind="ExternalOutput").ap()

    xin_t = xin.rearrange("p (t pp) d -> p pp t d", pp=P)  # [np, 128, NT, 64]
    out_t = out.rearrange("p (t pp) d -> p pp t d", pp=P)

    with tile.TileContext(nc) as tc:
        with (
            tc.tile_pool(name="singles", bufs=1) as singles,
            tc.tile_pool(name="stats", bufs=2) as stats,
            tc.tile_pool(name="tmp", bufs=3) as tmp,
            tc.tile_pool(name="epool", bufs=3) as epool,
            tc.tile_pool(name="fin", bufs=2) as fin,
            tc.tile_pool(name="ps_s", bufs=2, space="PSUM") as ps_s,
            tc.tile_pool(name="ps_u", bufs=1, space="PSUM") as ps_u,
        ):
            ident16 = singles.tile([P, P], BF16, tag="ident16")
            make_identity(nc, ident16[:])
            identf = singles.tile([P, P], F32, tag="identf")
            make_identity(nc, identf[:])

            # W^T in bf16 at both partition row groups (per block).
            wt16 = []
            for i in range(nblocks):
                wtmp = singles.tile([D, D], F32, tag=f"wtmp{i}", name=f"wtmp{i}")
                nc.sync.dma_start(wtmp[:], wts[i])
                w16 = singles.tile([P, D], BF16, tag=f"w16_{i}", name=f"w16_{i}")
                nc.vector.tensor_copy(w16[0:D, :], wtmp[:])
                nc.vector.tensor_copy(w16[D:P, :], wtmp[:])
                wt16.append(w16)

            # Per-pair xnt, duplicated across both partition halves so the
            # S-matmuls can run as two concurrent 64x64-quadrant streams
            # (tile (0,0) for S-rows 0-63, tile (64,64) for rows 64-127).
            wpk = {}
            for p in range(npairs):
                wpk[p] = singles.tile(
                    [P, n_rows], BF16, tag=f"wpk{p}", name=f"wpk{p}"
                )

            # m-packs: [128, NJP, 2, 80] fp8 (DoubleRow stationary) and the
            # bf16 twin for DVE-exp J-pairs.
            m8 = {}
            for p in range(npairs):
                m8[p] = singles.tile([P, NJP, 2, MW], FP8, tag=f"m8_{p}", name=f"m8_{p}")
                nc.vector.memset(m8[p][:, :, :, D : D + 1], 1.0)
                nc.vector.memset(m8[p][:, :, :, D + 1 : MW], 0.0)

            # per-block activation bias (-scale) as [128,1] const APs
            bias_t = []
            for i in range(nblocks):
                bt = singles.tile([P, 1], F32, tag=f"bias{i}", name=f"bias{i}")
                nc.vector.memset(bt[:], -float(scales[i]))
                bias_t.append(bt)

            # block inputs, row-major fp32
            xb = {}
            for p in range(npairs):
                for blk in range(nblocks):
                    xb[p, blk] = singles.tile(
                        [P, NT, D], F32, tag=f"xb_{p}_{blk}", name=f"xb_{p}_{blk}"
                    )
            for p in range(npairs):
                nc.sync.dma_start(xb[p, 0][:], xin_t[p])

            MAGIC = 0x5F3759DF

            def prep(p, blk):
                """norms -> 1/||x||, normalized bf16 rows transposed into
                wpack+mov, value pack m = [xW^T, 1] in fp8. DVE work is
                batched across all NT row-tiles per op."""
                xv = xb[p, blk]
                sq = tmp.tile([P, NT, D], F32, tag="sqb")
                nc.vector.tensor_mul(sq[:], xv[:], xv[:])
                s3 = stats.tile([P, NT, 1], F32, tag="s_all")
                nc.vector.reduce_sum(s3[:], sq[:], axis=mybir.AxisListType.X)
                nc.vector.tensor_scalar_max(s3[:], s3[:], 1e-24)
                # rinv = s^-0.5 fast inverse sqrt + 3 Newton steps
                r3 = stats.tile([P, NT, 1], F32, tag="rinv")
                s_i = s3[:].bitcast(I32)
                r_i = r3[:].bitcast(I32)
                nc.vector.tensor_scalar(
                    out=r_i, in0=s_i, scalar1=1, scalar2=None,
                    op0=ALU.logical_shift_right,
                )
                nc.vector.tensor_scalar(
                    out=r_i, in0=r_i, scalar1=MAGIC, scalar2=None, op0=ALU.subtract,
                )
                nc.vector.tensor_scalar(
                    out=r_i, in0=r_i, scalar1=-1, scalar2=None, op0=ALU.bitwise_xor,
                )
                nc.vector.tensor_scalar(
                    out=r_i, in0=r_i, scalar1=1, scalar2=None, op0=ALU.add,
                )
                t1 = stats.tile([P, NT, 1], F32, tag="nt1")
                for _ in range(3):
                    nc.vector.tensor_mul(t1[:], r3[:], r3[:])
                    nc.vector.tensor_mul(t1[:], t1[:], s3[:])
                    nc.vector.tensor_scalar(
                        out=t1[:], in0=t1[:], scalar1=-0.5, scalar2=1.5,
                        op0=ALU.mult, op1=ALU.add,
                    )
                    nc.vector.tensor_mul(r3[:], r3[:], t1[:])
                # ||x|| = s * r
                nrm3 = stats.tile([P, NT, 1], F32, tag="nrm")
                nc.vector.tensor_mul(nrm3[:], s3[:], r3[:])

                # normalize all rows in one broadcast op
                xn16a = tmp.tile([P, NT, D], BF16, tag="xn16a")
                b0, b1 = broadcast_tensor_aps(xv[:], r3[:])
                nc.vector.tensor_tensor(out=xn16a[:], in0=b0, in1=b1, op=ALU.mult)

                # PE transpose into wpk[p][0:64]; duplicate to rows 64-127
                # via one SBUF->SBUF DMA (DMA engines are idle).
                GJ = 8
                for g in range(NT // GJ):
                    pst = ps_s.tile([P, GJ * P], BF16, tag="S")
                    for k in range(GJ):
                        b = g * GJ + k
                        nc.tensor.transpose(
                            pst[0:D, k * P : (k + 1) * P],
                            xn16a[:, b, :], ident16[:],
                        )
                    cols = slice(g * GJ * P, (g + 1) * GJ * P)
                    nc.vector.tensor_copy(wpk[p][0:D, cols], pst[0:D, :])
                nc.sync.dma_start(wpk[p][D:P, :], wpk[p][0:D, :])
                # value pack: xW rows = (xn @ W^T) * ||x||, quantized to fp8
                for jp in range(NJP):
                    psw = ps_s.tile([P, 2, D], F32, tag="S")
                    for sl in range(2):
                        b = 2 * jp + sl
                        nc.tensor.matmul(
                            psw[:, sl, :],
                            lhsT=wpk[p][0:D, b * P : (b + 1) * P],
                            rhs=wt16[blk][0:D, :],
                            start=True, stop=True,
                        )
                    c0, c1 = broadcast_tensor_aps(
                        psw[:], nrm3[:, 2 * jp : 2 * jp + 2, :]
                    )
                    nc.vector.tensor_tensor(
                        out=m8[p][:, jp, :, 0:D], in0=c0, in1=c1, op=ALU.mult
                    )

            def main(blk, scale, last):
                bias_ap = bias_t[blk]
                # DVE Schraudolph: fp8e4 bits of 2^(log2e*scale*(s-1))
                # ~= int8(A8*s + B8)
                A8 = 8.0 * LOG2E * scale
                B8 = 8.0 * (7.0 - LOG2E * scale) + SCHRAUD_C * 8.0 / 128.0

                def use_dve(p, jp):
                    # ~7/16 of exp tiles on DVE, pairs alternating so ACT and
                    # DVE run concurrently within each J step.
                    return p == (jp % 2) and jp % 4 != 3

                for a in range(NCH):
                    U = {}
                    for p in range(npairs):
                        U[p] = ps_u.tile([MW, CHW], F32, tag=f"U{p}", name=f"U_{blk}_{a}_{p}")
                    for jp in range(NJP):
                        E = {}
                        for p in range(npairs):
                            E[p] = epool.tile(
                                [P, 2, CHW], FP8, tag=f"E{p}",
                                name=f"E_{blk}_{a}_{jp}_{p}",
                            )
                        for sl in range(2):
                            J = 2 * jp + sl
                            for p in range(npairs):
                                S = ps_s.tile([P, CHW], F32, tag="S")
                                for h in range(NH):
                                    c0 = a * CHW + h * HALF
                                    nc.tensor.matmul(
                                        S[0:D, h * HALF : (h + 1) * HALF],
                                        lhsT=wpk[p][0:D, J * P : J * P + D],
                                        rhs=wpk[p][0:D, c0 : c0 + HALF],
                                        start=True, stop=True,
                                    )
                                    nc.tensor.matmul(
                                        S[D:P, h * HALF : (h + 1) * HALF],
                                        lhsT=wpk[p][D:P, J * P + D : (J + 1) * P],
                                        rhs=wpk[p][D:P, c0 : c0 + HALF],
                                        start=True, stop=True,
                                    )
                                if use_dve(p, jp):
                                    nc.vector.tensor_scalar(
                                        out=E[p][:, sl, :].bitcast(mybir.dt.int8),
                                        in0=S[:],
                                        scalar1=A8, scalar2=B8,
                                        op0=ALU.mult, op1=ALU.add,
                                    )
                                else:
                                    nc.scalar.activation(
                                        E[p][:, sl, :], S[:], AF.Exp,
                                        scale=scale, bias=bias_ap[:],
                                    )
                        for p in range(npairs):
                            for h in range(NH):
                                nc.tensor.matmul(
                                    U[p][:, h * HALF : (h + 1) * HALF],
                                    lhsT=m8[p][:, jp],
                                    rhs=E[p][:, :, h * HALF : (h + 1) * HALF],
                                    start=(jp == 0), stop=(jp == NJP - 1),
                                    perf_mode=PM.DoubleRow,
                                    skip_group_check=True,
                                )
                    # chunk epilogue: transpose [G;Z] tiles (bf16),
                    # batched out = relu(G/Z + x) over 4 tiles per DVE op
                    for p in range(npairs):
                        GZ = fin.tile([D + 1, CHW], BF16, tag="GZ")
                        nc.vector.tensor_copy(GZ[:], U[p][0 : D + 1, :])
                        for g in range(CHW // P // 8):
                            T8 = ps_s.tile([P, 8, D + 2], BF16, tag="S")
                            for t8 in range(8):
                                t = g * 8 + t8
                                nc.tensor.transpose(
                                    T8[:, t8, 0 : D + 1],
                                    GZ[:, t * P : (t + 1) * P],
                                    ident16[0 : D + 1, 0 : D + 1],
                                )
                            gi0 = a * (CHW // P) + g * 8
                            rz8 = tmp.tile([P, 8, 1], F32, tag="rz")
                            nc.vector.reciprocal(rz8[:], T8[:, :, D : D + 1])
                            y8 = tmp.tile([P, 8, D], F32, tag="y4")
                            e0, e1 = broadcast_tensor_aps(T8[:, :, 0:D], rz8[:])
                            nc.vector.tensor_tensor(
                                out=y8[:], in0=e0, in1=e1, op=ALU.mult
                            )
                            nc.vector.tensor_add(
                                y8[:], y8[:], xb[p, blk][:, gi0 : gi0 + 8, :]
                            )
                            if not last:
                                nc.vector.tensor_scalar_max(
                                    xb[p, blk + 1][:, gi0 : gi0 + 8, :], y8[:], 0.0
                                )
                            else:
                                oo = tmp.tile([P, 8, D], F32, tag="oo")
                                nc.vector.tensor_scalar_max(oo[:], y8[:], 0.0)
                                nc.sync.dma_start(
                                    out_t[p][:, gi0 : gi0 + 8, :], oo[:]
                                )

            for blk in range(nblocks):
                for p in range(npairs):
                    prep(p, blk)
                main(blk, scales[blk], last=(blk == nblocks - 1))

    nc.compile()
    return nc


_CACHE = {}


def _get_nc(scales, n_rows, npairs):
    key = (tuple(scales), n_rows, npairs)
    if key not in _CACHE:
        _CACHE[key] = build_nc(list(scales), n_rows=n_rows, npairs=npairs)
    return _CACHE[key]


def kernel(x, W1, W2, alpha1, alpha2):
    x = np.asarray(x, dtype=np.float32)
    B, H, N, d = x.shape
    assert d == D and (B * H) % N_CORES == 0
    npairs = (B * H) // N_CORES
    s1 = 1.0 / max(float(alpha1), 0.01)
    s2 = 1.0 / max(float(alpha2), 0.01)
    nc = _get_nc((s1, s2), N, npairs)

    xf = np.ascontiguousarray(x.reshape(B * H, N, d))
    w0 = np.ascontiguousarray(np.asarray(W1, dtype=np.float32).T)
    w1 = np.ascontiguousarray(np.asarray(W2, dtype=np.float32).T)
    in_maps = [
        {"xin": xf[npairs * c : npairs * (c + 1)], "w0t": w0, "w1t": w1}
        for c in range(N_CORES)
    ]
    res = run_bass_kernel_spmd(nc, in_maps, core_ids=list(range(N_CORES)))
    outs = np.stack([r["out"] for r in res.results])
    return outs.reshape(B, H, N, d).astype(np.float32)


# revision 11
# speedup vs baseline: 1.1053x; 1.1053x over previous
"""Trainium2 Bass kernel for a 2-layer cosine-similarity attention GCN.

Reference math (per (b,h) slice, two chained blocks):
    xn = x / max(||x||_row, eps)
    A  = softmax((xn @ xn^T) / max(alpha, 0.01), axis=-1)
    out = relu((A @ x) @ W^T + x)

Shapes: x [4, 4, 4096, 64] fp32; W [64, 64]. B*H = 16 slices sharded as
2 slices per NeuronCore across 8 cores (fully independent, no collectives).

Architecture (per core, 2 pairs x 2 blocks, all on-chip):
  - (A @ x) @ W^T == A @ (x @ W^T): W is folded into the value matrix up
    front, so the per-chunk epilogue is just transpose + rescale + residual.
  - Softmax without max-subtraction (logits are cosine sims * scale):
    E = exp(s*scale - scale) in (0, 1]; Z arrives via a ones-column in the
    value pack; division by Z is applied after the epilogue transpose.
  - S-matmuls run with K=128 at full PE stream rate (0.42ns/col) by packing
    both pairs' normalized-transposed rows into one 128-row stationary
    (rows 0-63 pair0, 64-127 pair1) and zero-padding the inactive pair's
    rows of the moving operand.  K=64 matmuls stream at only half rate, so
    this doubles S throughput.
  - U-matmuls (E^T-weighted value sums) use fp8e4 DoubleRow: two J-tiles
    (contraction 256) per instruction at 0.42ns/col -> 2x over bf16.
    Value pack m = [x@W^T, 1, 0-pad] is 80 wide (DoubleRow needs the slot
    stride %16 == 0 and an even width).
  - exp is split between the scalar engine (ACT, fp8e4 out) and the vector
    engine: DVE computes exp via a one-op Schraudolph trick (y = s*A + B
    converted to int16 == bf16 bit pattern of 2^(log2e*(s*scale-scale))),
    emitting bf16 E tiles consumed by plain bf16 U-matmuls. This roughly
    halves the scalar-engine wall which otherwise bottlenecks the kernel.
  - row 1/||x|| uses a fast inverse sqrt (bit trick + 3 Newton steps) on the
    vector engine so the scalar engine only ever runs Exp.
"""

import numpy as np

import concourse.bacc as bacc
import concourse.tile as tile
from concourse import mybir
from concourse.bass import broadcast_tensor_aps
from concourse.bass_utils import run_bass_kernel_spmd
from concourse.masks import make_identity

F32 = mybir.dt.float32
U32 = mybir.dt.uint32
I32 = mybir.dt.int32
I16 = mybir.dt.int16
BF16 = mybir.dt.bfloat16
FP8 = mybir.dt.float8e4
AF = mybir.ActivationFunctionType
ALU = mybir.AluOpType
PM = mybir.MatmulPerfMode

P = 128
D = 64
N_CORES = 8
MW = 80  # m-pack width: 64 values + 1 ones + 15 zero pad (DoubleRow align)
LOG2E = 1.4426950408889634
# Schraudolph correction picked to roughly center the relative error of
# 2^f ~ linear-in-mantissa; in bf16-bit units (128 per octave).
SCHRAUD_C = -9.3

# J-pair indices whose exp runs on the DVE (bf16 E) instead of ACT (fp8 E).
DVE_JP = (2, 6, 10, 14)


def build_nc(scales, n_rows=4096, npairs=2):
    assert npairs == 2, "kernel assumes 2 (b,h) slices per core"
    nblocks = len(scales)
    NT = n_rows // P          # 32 row tiles per pair
    CHW = 1024                # i-chunk width (ACT call width, 2 PSUM banks)
    NCH = n_rows // CHW       # 4 chunks
    HALF = 512                # matmul moving-dim limit
    NH = CHW // HALF
    NJP = NT // 2             # 16 J-pairs

    nc = bacc.Bacc("TRN2", target_bir_lowering=False, debug=False, num_devices=N_CORES)
    xin = nc.dram_tensor("xin", [npairs, n_rows, D], F32, kind="ExternalInput").ap()
    wts = [
        nc.dram_tensor(f"w{i}t", [D, D], F32, kind="ExternalInput").ap()
        for i in range(nblocks)
    ]
    out = nc.dram_tensor("out", [npairs, n_rows, D], F32, kind="ExternalOutput").ap()

    xin_t = xin.rearrange("p (t pp) d -> p pp t d", pp=P)  # [np, 128, NT, 64]
    out_t = out.rearrange("p (t pp) d -> p pp t d", pp=P)

    with tile.TileContext(nc) as tc:
        with (
            tc.tile_pool(name="singles", bufs=1) as singles,
            tc.tile_pool(name="stats", bufs=2) as stats,
            tc.tile_pool(name="tmp", bufs=3) as tmp,
            tc.tile_pool(name="epool", bufs=3) as epool,
            tc.tile_pool(name="fin", bufs=2) as fin,
            tc.tile_pool(name="ps_s", bufs=2, space="PSUM") as ps_s,
            tc.tile_pool(name="ps_u", bufs=1, space="PSUM") as ps_u,
        ):
            ident16 = singles.tile([P, P], BF16, tag="ident16")
            make_identity(nc, ident16[:])
            identf = singles.tile([P, P], F32, tag="identf")
            make_identity(nc, identf[:])

            # W^T in bf16 at both partition row groups (per block).
            wt16 = []
            for i in range(nblocks):
                wtmp = singles.tile([D, D], F32, tag=f"wtmp{i}", name=f"wtmp{i}")
                nc.sync.dma_start(wtmp[:], wts[i])
                w16 = singles.tile([P, D], BF16, tag=f"w16_{i}", name=f"w16_{i}")
                nc.vector.tensor_copy(w16[0:D, :], wtmp[:])
                nc.vector.tensor_copy(w16[D:P, :], wtmp[:])
                wt16.append(w16)

            # Persistent layouts (reused across blocks):
            #  wpack [128, N]: rows 0-63 pair0 xnt, rows 64-127 pair1 xnt
            #  mov [128, 2N]: cols [0,N) pair0 data / pair1 rows zeroed;
            #                 cols [N,2N) pair1 data / pair0 rows zeroed.
            #  K=128 matmuls stream 2x faster than K=64, so padding the
            #  inactive pair's rows with zeros doubles S throughput.
            wpack = singles.tile([P, n_rows], BF16, tag="wpack")
            mov = singles.tile([P, 2 * n_rows], BF16, tag="mov")
            nc.vector.memset(mov[D:P, 0:n_rows], 0.0)
            nc.vector.memset(mov[0:D, n_rows : 2 * n_rows], 0.0)

            # m-packs: [128, NJP, 2, 80] fp8 (DoubleRow stationary) and the
            # bf16 twin for DVE-exp J-pairs.
            m8 = {}
            for p in range(npairs):
                m8[p] = singles.tile([P, NJP, 2, MW], FP8, tag=f"m8_{p}", name=f"m8_{p}")
                nc.vector.memset(m8[p][:, :, :, D : D + 1], 1.0)
                nc.vector.memset(m8[p][:, :, :, D + 1 : MW], 0.0)

            # per-block activation bias (-scale) as [128,1] const APs
            bias_t = []
            for i in range(nblocks):
                bt = singles.tile([P, 1], F32, tag=f"bias{i}", name=f"bias{i}")
                nc.vector.memset(bt[:], -float(scales[i]))
                bias_t.append(bt)

            # block inputs, row-major fp32
            xb = {}
            for p in range(npairs):
                for blk in range(nblocks):
                    xb[p, blk] = singles.tile(
                        [P, NT, D], F32, tag=f"xb_{p}_{blk}", name=f"xb_{p}_{blk}"
                    )
            for p in range(npairs):
                nc.sync.dma_start(xb[p, 0][:], xin_t[p])

            MAGIC = 0x5F3759DF

            def prep(p, blk):
                """norms -> 1/||x||, normalized bf16 rows transposed into
                wpack+mov, value pack m = [xW^T, 1] in fp8. DVE work is
                batched across all NT row-tiles per op."""
                lo = D * p
                xv = xb[p, blk]
                sq = tmp.tile([P, NT, D], F32, tag="sqb")
                nc.vector.tensor_mul(sq[:], xv[:], xv[:])
                s3 = stats.tile([P, NT, 1], F32, tag="s_all")
                nc.vector.reduce_sum(s3[:], sq[:], axis=mybir.AxisListType.X)
                nc.vector.tensor_scalar_max(s3[:], s3[:], 1e-24)
                # rinv = s^-0.5 fast inverse sqrt + 3 Newton steps
                r3 = stats.tile([P, NT, 1], F32, tag="rinv")
                s_i = s3[:].bitcast(I32)
                r_i = r3[:].bitcast(I32)
                nc.vector.tensor_scalar(
                    out=r_i, in0=s_i, scalar1=1, scalar2=None,
                    op0=ALU.logical_shift_right,
                )
                nc.vector.tensor_scalar(
                    out=r_i, in0=r_i, scalar1=MAGIC, scalar2=None, op0=ALU.subtract,
                )
                nc.vector.tensor_scalar(
                    out=r_i, in0=r_i, scalar1=-1, scalar2=None, op0=ALU.bitwise_xor,
                )
                nc.vector.tensor_scalar(
                    out=r_i, in0=r_i, scalar1=1, scalar2=None, op0=ALU.add,
                )
                t1 = stats.tile([P, NT, 1], F32, tag="nt1")
                for _ in range(3):
                    nc.vector.tensor_mul(t1[:], r3[:], r3[:])
                    nc.vector.tensor_mul(t1[:], t1[:], s3[:])
                    nc.vector.tensor_scalar(
                        out=t1[:], in0=t1[:], scalar1=-0.5, scalar2=1.5,
                        op0=ALU.mult, op1=ALU.add,
                    )
                    nc.vector.tensor_mul(r3[:], r3[:], t1[:])
                # ||x|| = s * r
                nrm3 = stats.tile([P, NT, 1], F32, tag="nrm")
                nc.vector.tensor_mul(nrm3[:], s3[:], r3[:])

                # normalize all rows in one broadcast op
                xn16a = tmp.tile([P, NT, D], BF16, tag="xn16a")
                b0, b1 = broadcast_tensor_aps(xv[:], r3[:])
                nc.vector.tensor_tensor(out=xn16a[:], in0=b0, in1=b1, op=ALU.mult)

                # PE transpose into wpack row group; mov copy via
                # SBUF->SBUF DMA (DMA engines are idle).
                GJ = 8
                for g in range(NT // GJ):
                    pst = ps_s.tile([P, GJ * P], BF16, tag="S")
                    for k in range(GJ):
                        b = g * GJ + k
                        nc.tensor.transpose(
                            pst[lo : lo + D, k * P : (k + 1) * P],
                            xn16a[:, b, :], ident16[:],
                        )
                    cols = slice(g * GJ * P, (g + 1) * GJ * P)
                    nc.vector.tensor_copy(
                        wpack[lo : lo + D, cols], pst[lo : lo + D, :]
                    )
                nc.sync.dma_start(
                    mov[lo : lo + D, p * n_rows : (p + 1) * n_rows],
                    wpack[lo : lo + D, :],
                )
                # value pack: xW rows = (xn @ W^T) * ||x||, quantized to fp8
                for jp in range(NJP):
                    psw = ps_s.tile([P, 2, D], F32, tag="S")
                    for sl in range(2):
                        b = 2 * jp + sl
                        nc.tensor.matmul(
                            psw[:, sl, :],
                            lhsT=wpack[lo : lo + D, b * P : (b + 1) * P],
                            rhs=wt16[blk][lo : lo + D, :],
                            start=True, stop=True,
                        )
                    c0, c1 = broadcast_tensor_aps(
                        psw[:], nrm3[:, 2 * jp : 2 * jp + 2, :]
                    )
                    nc.vector.tensor_tensor(
                        out=m8[p][:, jp, :, 0:D], in0=c0, in1=c1, op=ALU.mult
                    )

            def main(blk, scale, last):
                bias_ap = bias_t[blk]
                # DVE Schraudolph: fp8e4 bits of 2^(log2e*scale*(s-1))
                # ~= int8(A8*s + B8)
                A8 = 8.0 * LOG2E * scale
                B8 = 8.0 * (7.0 - LOG2E * scale) + SCHRAUD_C * 8.0 / 128.0

                def use_dve(p, jp):
                    # ~7/16 of exp tiles on DVE, pairs alternating so ACT and
                    # DVE run concurrently within each J step.
                    return p == (jp % 2) and jp % 4 != 3

                for a in range(NCH):
                    U = {}
                    for p in range(npairs):
                        U[p] = ps_u.tile([MW, CHW], F32, tag=f"U{p}", name=f"U_{blk}_{a}_{p}")
                    for jp in range(NJP):
                        E = {}
                        for p in range(npairs):
                            E[p] = epool.tile(
                                [P, 2, CHW], FP8, tag=f"E{p}",
                                name=f"E_{blk}_{a}_{jp}_{p}",
                            )
                        for sl in range(2):
                            J = 2 * jp + sl
                            # emit the ACT pair's S first so the scalar
                            # engine starts as early as possible
                            order = sorted(
                                range(npairs), key=lambda q: use_dve(q, jp)
                            )
                            for p in order:
                                S = ps_s.tile([P, CHW], F32, tag="S")
                                for h in range(NH):
                                    nc.tensor.matmul(
                                        S[:, h * HALF : (h + 1) * HALF],
                                        lhsT=wpack[:, J * P : (J + 1) * P],
                                        rhs=mov[
                                            :,
                                            p * n_rows + a * CHW + h * HALF :
                                            p * n_rows + a * CHW + (h + 1) * HALF,
                                        ],
                                        start=True, stop=True,
                                    )
                                if use_dve(p, jp):
                                    nc.vector.tensor_scalar(
                                        out=E[p][:, sl, :].bitcast(mybir.dt.int8),
                                        in0=S[:],
                                        scalar1=A8, scalar2=B8,
                                        op0=ALU.mult, op1=ALU.add,
                                    )
                                else:
                                    nc.scalar.activation(
                                        E[p][:, sl, :], S[:], AF.Exp,
                                        scale=scale, bias=bias_ap[:],
                                    )
                        for p in range(npairs):
                            for h in range(NH):
                                nc.tensor.matmul(
                                    U[p][:, h * HALF : (h + 1) * HALF],
                                    lhsT=m8[p][:, jp],
                                    rhs=E[p][:, :, h * HALF : (h + 1) * HALF],
                                    start=(jp == 0), stop=(jp == NJP - 1),
                                    perf_mode=PM.DoubleRow,
                                    skip_group_check=True,
                                )
                    # chunk epilogue: transpose [G;Z] tiles (bf16),
                    # batched out = relu(G/Z + x) over 4 tiles per DVE op
                    for p in range(npairs):
                        GZ = fin.tile([D + 1, CHW], BF16, tag="GZ")
                        nc.vector.tensor_copy(GZ[:], U[p][0 : D + 1, :])
                        for g in range(CHW // P // 8):
                            T8 = ps_s.tile([P, 8, D + 2], BF16, tag="S")
                            for t8 in range(8):
                                t = g * 8 + t8
                                nc.tensor.transpose(
                                    T8[:, t8, 0 : D + 1],
                                    GZ[:, t * P : (t + 1) * P],
                                    ident16[0 : D + 1, 0 : D + 1],
                                )
                            gi0 = a * (CHW // P) + g * 8
                            rz8 = tmp.tile([P, 8, 1], F32, tag="rz")
                            nc.vector.reciprocal(rz8[:], T8[:, :, D : D + 1])
                            y8 = tmp.tile([P, 8, D], F32, tag="y4")
                            e0, e1 = broadcast_tensor_aps(T8[:, :, 0:D], rz8[:])
                            nc.vector.tensor_tensor(
                                out=y8[:], in0=e0, in1=e1, op=ALU.mult
                            )
                            nc.vector.tensor_add(
                                y8[:], y8[:], xb[p, blk][:, gi0 : gi0 + 8, :]
                            )
                            if not last:
                                nc.vector.tensor_scalar_max(
                                    xb[p, blk + 1][:, gi0 : gi0 + 8, :], y8[:], 0.0
                                )
                            else:
                                oo = tmp.tile([P, 8, D], F32, tag="oo")
                                nc.vector.tensor_scalar_max(oo[:], y8[:], 0.0)
                                nc.sync.dma_start(
                                    out_t[p][:, gi0 : gi0 + 8, :], oo[:]
                                )

            for blk in range(nblocks):
                for p in range(npairs):
                    prep(p, blk)
                main(blk, scales[blk], last=(blk == nblocks - 1))

    nc.compile()
    return nc


_CACHE = {}


def _get_nc(scales, n_rows, npairs):
    key = (tuple(scales), n_rows, npairs)
    if key not in _CACHE:
        _CACHE[key] = build_nc(list(scales), n_rows=n_rows, npairs=npairs)
    return _CACHE[key]


def kernel(x, W1, W2, alpha1, alpha2):
    x = np.asarray(x, dtype=np.float32)
    B, H, N, d = x.shape
    assert d == D and (B * H) % N_CORES == 0
    npairs = (B * H) // N_CORES
    s1 = 1.0 / max(float(alpha1), 0.01)
    s2 = 1.0 / max(float(alpha2), 0.01)
    nc = _get_nc((s1, s2), N, npairs)

    xf = np.ascontiguousarray(x.reshape(B * H, N, d))
    w0 = np.ascontiguousarray(np.asarray(W1, dtype=np.float32).T)
    w1 = np.ascontiguousarray(np.asarray(W2, dtype=np.float32).T)
    in_maps = [
        {"xin": xf[npairs * c : npairs * (c + 1)], "w0t": w0, "w1t": w1}
        for c in range(N_CORES)
    ]
    res = run_bass_kernel_spmd(nc, in_maps, core_ids=list(range(N_CORES)))
    outs = np.stack([r["out"] for r in res.results])
    return outs.reshape(B, H, N, d).astype(np.float32)
